# revision 57
# baseline (speedup 1.0000x reference)
"""CrossAttentionFusion forward on 8 Trainium2 NeuronCores (pure data parallel).

Math folded on host (seq-len-1 MHA == two chained linears):
  d_att = micro @ A_dm + c_dm,  A_dm = Wv_dm.T @ Wout_dm.T
  m_att = drug  @ A_md + c_md
  u = drug + d_att ; w = micro + m_att
  xu = (u - mu)/sd ; xw likewise        (LN affine folded into W1)
  h1 = gelu([xu, xw] @ W1f + b1f),  W1f = (ffn_w1 * g_cat).T
  h2 = h1 @ W2f + b2,               W2f = ffn_w2.T
  out = ((h2 - mu)/sd) * g_out + b_out

Device layout: feature-major [feat(partition), batch(free)], batch sharded
across 8 cores, NB=256 batch columns per tile (32 tiles/core).

Software pipeline per iteration t (PE program order):
  attn(t) -> out-LN(t-2) -> FFN1(t-1) -> LN-stats(t) -> FFN2(t-1)
so the cross-engine LayerNorm stat chains (PE stats -> DVE smalls ->
GpSimd partition-broadcast -> DVE normalize) always have ~8us of
independent PE work in front of their consumers.  LN mean/rstd are
broadcast across partitions by GpSimd (partition_broadcast), not by PE
matmuls, freeing PE cycles and PSUM banks.
"""

import sys

if "/opt/trn_rl_repo" not in sys.path:
    sys.path.insert(0, "/opt/trn_rl_repo")

from contextlib import ExitStack

import ml_dtypes
import numpy as np

import concourse.bass as bass  # noqa: F401  (registers mybir lowering hooks)
import concourse.tile as tile
from concourse import bacc, mybir
from concourse.bass import ts, broadcast_tensor_aps
from concourse.bass_utils import run_bass_kernel_spmd

F32 = mybir.dt.float32
BF16 = mybir.dt.bfloat16
FP8 = mybir.dt.float8e4
DR = mybir.MatmulPerfMode.DoubleRow
ACT = mybir.ActivationFunctionType

P = 128
D = 384
KD = D // P          # 3
DH = 2 * D           # 768
KH = DH // P         # 6
DF = 4 * D           # 1536
KF = DF // P         # 12
EPS = 1e-5
N_CORES = 8
B_FULL = 65536
BC = B_FULL // N_CORES   # 8192 rows per core
NB = 512                 # batch columns per on-chip tile
NT = BC // NB            # 32 tiles per core

_NC_CACHE = {}
LAST_RESULTS = None      # BassKernelResults of the most recent kernel() call


def _build_nc(bc, flags):
    use_c_dm, use_c_md, use_b1, use_b2, use_affine = flags
    nt = bc // NB
    nc = bacc.Bacc("TRN2", target_bir_lowering=False, debug=False,
                   num_devices=N_CORES)

    xd_d = nc.dram_tensor("xd", [D, bc], BF16, kind="ExternalInput")
    xm_d = nc.dram_tensor("xm", [D, bc], BF16, kind="ExternalInput")
    mx_d = nc.dram_tensor("mx", [1, 2, bc], BF16, kind="ExternalInput")
    a_dm_d = nc.dram_tensor("a_dm", [D, D], BF16, kind="ExternalInput")
    a_md_d = nc.dram_tensor("a_md", [D, D], BF16, kind="ExternalInput")
    w1_d = nc.dram_tensor("w1", [DH, DF], BF16, kind="ExternalInput")
    w2_d = nc.dram_tensor("w2", [DF, D], BF16, kind="ExternalInput")
    c_dm_d = nc.dram_tensor("c_dm", [D], F32, kind="ExternalInput") if use_c_dm else None
    c_md_d = nc.dram_tensor("c_md", [D], F32, kind="ExternalInput") if use_c_md else None
    b1_d = nc.dram_tensor("b1", [DF], F32, kind="ExternalInput") if use_b1 else None
    b2_d = nc.dram_tensor("b2", [D], F32, kind="ExternalInput") if use_b2 else None
    g_o_d = nc.dram_tensor("g_o", [D], F32, kind="ExternalInput") if use_affine else None
    b_o_d = nc.dram_tensor("b_o", [D], F32, kind="ExternalInput") if use_affine else None
    o_d = nc.dram_tensor("o", [D, bc], BF16, kind="ExternalOutput")

    xd_r = xd_d.ap().rearrange("(k p) n -> p k n", p=P)
    xm_r = xm_d.ap().rearrange("(k p) n -> p k n", p=P)
    o_r = o_d.ap().rearrange("(k p) n -> p k n", p=P)

    with tile.TileContext(nc) as tc, ExitStack() as ctx:
        wp = ctx.enter_context(tc.tile_pool(name="wts", bufs=1))
        xp = ctx.enter_context(tc.tile_pool(name="x", bufs=4))
        up = ctx.enter_context(tc.tile_pool(name="u", bufs=3))
        sqp = ctx.enter_context(tc.tile_pool(name="sq", bufs=2))
        xhp = ctx.enter_context(tc.tile_pool(name="xh", bufs=2))
        h1p = ctx.enter_context(tc.tile_pool(name="h1", bufs=2))
        h2p = ctx.enter_context(tc.tile_pool(name="h2", bufs=3))
        op_ = ctx.enter_context(tc.tile_pool(name="o", bufs=2))
        stp = ctx.enter_context(tc.tile_pool(name="st", bufs=2))
        bcp = ctx.enter_context(tc.tile_pool(name="bc", bufs=2))
        # PSUM: 6 single-bank [P,NB] tiles for all matmul outputs
        # (attn 6, ffn1 12, ffn2 3 allocs/iter) + 2 stats banks = 8.
        pp = ctx.enter_context(tc.tile_pool(name="pp", bufs=6, space="PSUM"))
        pst = ctx.enter_context(tc.tile_pool(name="pst", bufs=1, space="PSUM"))

        a_dm_sb = wp.tile([P, KD, D], BF16)
        nc.gpsimd.dma_start(a_dm_sb[:], a_dm_d.ap().rearrange("(k p) m -> p k m", p=P))
        a_md_sb = wp.tile([P, KD, D], BF16)
        nc.gpsimd.dma_start(a_md_sb[:], a_md_d.ap().rearrange("(k p) m -> p k m", p=P))
        # w1/w2 are large (~3.5MB); their DMA emission is deferred into the
        # loop (t==0 tail) so the first tile's mean-broadcast isn't queued
        # behind them on the GpSimd queue.
        w1_sb = wp.tile([P, KH, DF], BF16)
        w2_sb = wp.tile([P, KF, D], BF16)

        def emit_big_weight_dmas():
            nc.gpsimd.dma_start(w1_sb[:], w1_d.ap().rearrange("(k p) m -> p k m", p=P))
            nc.gpsimd.dma_start(w2_sb[:], w2_d.ap().rearrange("(k p) m -> p k m", p=P))

        ones16 = wp.tile([P, 1], BF16)
        nc.vector.memset(ones16[:], 1.0)
        # fp8 DoubleRow "ones" weights for the sum-of-squares stat matmuls:
        # lhsT [P, 2(k-pair), 16(M padded for the 16B step rule)], col 0 = 1.
        ones8 = wp.tile([P, 2, 16], FP8)
        nc.vector.memset(ones8[:], 0.0)
        nc.vector.memset(ones8[:, :, 0:1], 1.0)
        ones8r = wp.tile([P, 1], FP8)
        nc.vector.memset(ones8r[:], 1.0)
        eps_sb = wp.tile([1, 1], F32)
        nc.vector.memset(eps_sb[:], EPS)

        def vec_const(dram, nk, tag):
            t = wp.tile([P, nk], F32, tag=tag)
            nc.gpsimd.dma_start(t[:], dram.ap().rearrange("(k p) -> p k", p=P))
            return t

        c_dm_sb = vec_const(c_dm_d, KD, "c_dm") if use_c_dm else None
        c_md_sb = vec_const(c_md_d, KD, "c_md") if use_c_md else None
        b1_sb = vec_const(b1_d, KF, "b1") if use_b1 else None
        b2_sb = vec_const(b2_d, KD, "b2") if use_b2 else None
        g_o_sb = vec_const(g_o_d, KD, "g_o") if use_affine else None
        b_o_sb = vec_const(b_o_d, KD, "b_o") if use_affine else None

        # Persistent stats PSUM: two [16,NB] slots (2 banks at NB=512; 16
        # partitions because fp8-DoubleRow output spans lhsT.free/2 = 16
        # rows, of which row 0 carries the sum).  Disjoint uses per
        # iteration: {ssq_o} early, {ssq_u, ssq_w} late.
        stat_ps = pst.tile([16, 2, NB], F32)

        def ssq_mms(slot, sq):
            """sum over 3 k-slots of sq (fp8): 1 DoubleRow + 1 regular MM."""
            nc.tensor.matmul(stat_ps[0:16, slot, :], ones8[:],
                             sq[:, 0:2, :], start=True, stop=False,
                             perf_mode=DR, skip_group_check=True)
            nc.tensor.matmul(stat_ps[0:1, slot, :], ones8r[:],
                             sq[:, 2, :], start=False, stop=True,
                             skip_group_check=True)

        def bsub(out_ap, a_ap, b_ap):
            a2, b2_ = broadcast_tensor_aps(a_ap, b_ap)
            o2, _ = broadcast_tensor_aps(out_ap, b_ap)
            nc.vector.tensor_sub(o2, a2, b2_)

        def bmul(out_ap, a_ap, b_ap):
            a2, b2_ = broadcast_tensor_aps(a_ap, b_ap)
            o2, _ = broadcast_tensor_aps(out_ap, b_ap)
            nc.vector.tensor_mul(o2, a2, b2_)

        # LN structure: attention matrices are row-mean-centered on the host
        # (A' = A - rowmean(A)), so mean(u) == colmean(xd) exactly; the means
        # themselves are precomputed on the host and DMA'd in.  Residuals are
        # centered BEFORE the attention output lands, the Squares of the
        # centered sums give the variance directly, and the only
        # latency-critical chain is
        #   ssq-matmuls -> 1/D scale -> Sqrt -> recip -> cast -> inv bcast
        #   -> 2 muls -> xh
        # ONE Sqrt instruction per iteration covers slots {0,1}=u/w(t) and
        # {2}=out-LN(t-2) (single Gelu<->Sqrt table round-trip).
        def rsqrt_chain(var3, lo, hi):
            sd = stp.tile([1, 3, NB], F32, tag="sd")
            nc.scalar.activation(sd[:, lo:hi, :], var3[:, lo:hi, :], ACT.Sqrt,
                                 bias=eps_sb[:])
            inv = stp.tile([1, 3, NB], F32, tag="iv")
            nc.vector.reciprocal_approx_fast(inv[:, lo:hi, :], sd[:, lo:hi, :])
            inv16 = stp.tile([1, 3, NB], BF16, tag="iv16")
            nc.vector.tensor_copy(inv16[:, lo:hi, :], inv[:, lo:hi, :])
            bc_uw = bc_o = None
            if lo == 0:
                bc_uw = bcp.tile([P, 2, NB], BF16, tag="ivbuw")
                nc.gpsimd.partition_broadcast(bc_uw[:, 0:1, :],
                                              inv16[:, 0:1, :])
                nc.gpsimd.partition_broadcast(bc_uw[:, 1:2, :],
                                              inv16[:, 1:2, :])
            if hi == 3:
                bc_o = bcp.tile([P, 1, NB], BF16, tag="ivbo")
                nc.gpsimd.partition_broadcast(bc_o[:], inv16[:, 2:3, :])
            return bc_uw, bc_o

        state = {}

        def emit_means(t, sl):
            xd = xp.tile([P, KD, NB], BF16, tag="xd")
            nc.sync.dma_start(xd[:], xd_r[:, :, sl])
            xm = xp.tile([P, KD, NB], BF16, tag="xm")
            nc.sync.dma_start(xm[:], xm_r[:, :, sl])
            mu_uw = stp.tile([1, 2, NB], BF16, tag="mu_uw")
            nc.sync.dma_start(mu_uw[:], mx_d.ap()[:, :, sl])
            mu_bc = bcp.tile([P, 2, NB], BF16, tag="mubuw")
            nc.gpsimd.partition_broadcast(mu_bc[:], mu_uw[:])
            # centered residuals, ready before the attention output lands
            xd_c = up.tile([P, KD, NB], BF16, tag="xdc")
            bsub(xd_c[:], xd[:], mu_bc[:, 0:1, :])
            xm_c = up.tile([P, KD, NB], BF16, tag="xmc")
            bsub(xm_c[:], xm[:], mu_bc[:, 1:2, :])
            state[t] = {"xd": xd, "xm": xm, "xd_c": xd_c, "xm_c": xm_c}

        def emit_attn(t):
            st_ = state[t]
            xd, xm = st_["xd"], st_["xm"]

            def attn(a_sb, rhs, res_c, c_sb, tag):
                v = up.tile([P, KD, NB], BF16, tag=tag)
                for m in range(KD):
                    ps = pp.tile([P, NB], F32, tag="pp")
                    for k in range(KD):
                        nc.tensor.matmul(ps[:],
                                         a_sb[:, k, ts(m, P)],
                                         rhs[:, k, :],
                                         start=(k == 0), stop=(k == KD - 1))
                    nc.vector.tensor_add(v[:, m, :], ps[:], res_c[:, m, :])
                    if c_sb is not None:
                        nc.vector.tensor_scalar_add(v[:, m, :], v[:, m, :],
                                                    c_sb[:, m:m + 1])
                sq = sqp.tile([P, KD, NB], FP8, tag="sq" + tag)
                nc.scalar.activation(sq[:], v[:], ACT.Square)
                return v, sq

            u, sq_u = attn(a_dm_sb, xm, st_["xd_c"], c_dm_sb, "u")
            w, sq_w = attn(a_md_sb, xd, st_["xm_c"], c_md_sb, "w")
            st_.update(u=u, w=w, sq_u=sq_u, sq_w=sq_w)

        def emit_uw_ssq(t, var3):
            st_ = state[t]
            ssq_mms(0, st_["sq_u"])
            ssq_mms(1, st_["sq_w"])
            nc.vector.tensor_scalar_mul(var3[:, 0:2, :], stat_ps[0:1, 0:2, :],
                                        float(1.0 / D))

        def emit_norm_uw(t, inv_bc):
            st_ = state[t]
            xh = xhp.tile([P, KH, NB], BF16, tag="xh")
            bmul(xh[:, 0:KD, :], st_["u"][:], inv_bc[:, 0:1, :])
            bmul(xh[:, KD:KH, :], st_["w"][:], inv_bc[:, 1:2, :])
            st_["xh"] = xh

        def emit_ffn1(t):
            st_ = state[t]
            xh = st_["xh"]
            h1 = h1p.tile([P, KF, NB], BF16, tag="h1")
            for m in range(KF):
                ps = pp.tile([P, NB], F32, tag="pp")
                for k in range(KH):
                    nc.tensor.matmul(ps[:],
                                     w1_sb[:, k, ts(m, P)],
                                     xh[:, k, :],
                                     start=(k == 0), stop=(k == KH - 1))
                if use_b1:
                    nc.scalar.activation(h1[:, m, :], ps[:],
                                         ACT.Gelu, bias=b1_sb[:, m:m + 1])
                else:
                    nc.scalar.activation(h1[:, m, :], ps[:], ACT.Gelu)
            st_["h1"] = h1

        def emit_ffn2(t):
            st_ = state[t]
            h1 = st_["h1"]
            h2 = h2p.tile([P, KD, NB], BF16, tag="h2")
            for m in range(KD):
                ps = pp.tile([P, NB], F32, tag="pp")
                for k in range(KF):
                    nc.tensor.matmul(ps[:], w2_sb[:, k, ts(m, P)],
                                     h1[:, k, :],
                                     start=(k == 0), stop=(k == KF - 1))
                if use_b2:
                    nc.scalar.activation(h2[:, m, :], ps[:],
                                         ACT.Identity, bias=b2_sb[:, m:m + 1])
                else:
                    nc.vector.tensor_copy(h2[:, m, :], ps[:])
            st_["h2"] = h2

        def emit_outln_stats(t, var3):
            # W2 is row-mean-centered on the host, so mean(h2) == 0 exactly:
            # no mean matmuls / broadcast / subtraction needed for the out-LN.
            st_ = state[t]
            h2 = st_["h2"]
            sq_o = sqp.tile([P, KD, NB], FP8, tag="sqo")
            nc.scalar.activation(sq_o[:], h2[:], ACT.Square)
            ssq_mms(1, sq_o)
            nc.vector.tensor_scalar_mul(var3[:, 2, :], stat_ps[0:1, 1, :],
                                        float(1.0 / D))

        def emit_outln_norm_store(t, inv_bo, sl):
            st_ = state.pop(t)
            h2 = st_["h2"]
            o = op_.tile([P, KD, NB], BF16, tag="o")
            bmul(o[:], h2[:], inv_bo[:])
            if use_affine:
                for k in range(KD):
                    nc.vector.tensor_scalar(o[:, k, :], o[:, k, :],
                                            g_o_sb[:, k:k + 1],
                                            b_o_sb[:, k:k + 1],
                                            mybir.AluOpType.mult,
                                            mybir.AluOpType.add)
            nc.sync.dma_start(o_r[:, :, sl], o[:])

        for t in range(nt + 2):
            sl = slice(t * NB, (t + 1) * NB)
            lo = 0 if t < nt else 2
            hi = 3 if t >= 2 else 2
            var3 = stp.tile([1, 3, NB], F32, tag="var3")
            if t < nt:
                emit_means(t, sl)
                emit_attn(t)
            if t == 0:
                emit_big_weight_dmas()
            if t >= 2:
                emit_outln_stats(t - 2, var3)
            if 1 <= t <= nt:
                emit_ffn1(t - 1)
            if t < nt:
                emit_uw_ssq(t, var3)
            bc_uw, bc_o = rsqrt_chain(var3, lo, hi)
            if t < nt:
                emit_norm_uw(t, bc_uw)
            if t >= 2:
                osl = slice((t - 2) * NB, (t - 1) * NB)
                emit_outln_norm_store(t - 2, bc_o, osl)
            if 1 <= t <= nt:
                emit_ffn2(t - 1)

    nc.compile()
    return nc


def kernel(**inputs) -> np.ndarray:
    global LAST_RESULTS
    f = lambda k: np.asarray(inputs[k], np.float32)

    drug = f("drug_emb")
    micro = f("micro_emb")
    b = drug.shape[0]
    bc = b // N_CORES
    assert b % (N_CORES * NB) == 0

    # ---- host-side weight folding ----
    wv_dm, bv_dm = f("dm_in_w")[2 * D:], f("dm_in_b")[2 * D:]
    wv_md, bv_md = f("md_in_w")[2 * D:], f("md_in_b")[2 * D:]
    # Row-mean-center the attention matrices (and mean-center the bias
    # vectors) so that on-device mean(u) == colmean(xd): the LN mean then
    # only depends on the raw input, not on the attention output.
    a_dm = wv_dm.T @ f("dm_out_w").T
    a_dm = np.ascontiguousarray(a_dm - a_dm.mean(1, keepdims=True)).astype(ml_dtypes.bfloat16)
    c_dm = bv_dm @ f("dm_out_w").T + f("dm_out_b")
    c_dm = c_dm - c_dm.mean()
    a_md = wv_md.T @ f("md_out_w").T
    a_md = np.ascontiguousarray(a_md - a_md.mean(1, keepdims=True)).astype(ml_dtypes.bfloat16)
    c_md = bv_md @ f("md_out_w").T + f("md_out_b")
    c_md = c_md - c_md.mean()
    g_cat = np.concatenate([f("norm_d_g"), f("norm_m_g")])
    b_cat = np.concatenate([f("norm_d_b"), f("norm_m_b")])
    w1f = np.ascontiguousarray((f("ffn_w1") * g_cat[None, :]).T).astype(ml_dtypes.bfloat16)
    b1f = f("ffn_b1") + b_cat @ f("ffn_w1").T
    # Row-mean-center W2 (and mean-center b2) so mean(h2) == 0 exactly and
    # the out-LN needs no mean computation on device.
    w2f = f("ffn_w2").T
    w2f = np.ascontiguousarray(w2f - w2f.mean(1, keepdims=True)).astype(ml_dtypes.bfloat16)
    b2 = f("ffn_b2")
    b2 = b2 - b2.mean()
    g_o, b_o = f("norm_out_g"), f("norm_out_b")

    flags = (bool(np.any(c_dm)), bool(np.any(c_md)), bool(np.any(b1f)),
             bool(np.any(b2)), bool(np.any(g_o != 1.0) or np.any(b_o)))

    key = (bc, NB, flags)
    if key not in _NC_CACHE:
        _NC_CACHE[key] = _build_nc(bc, flags)
    nc = _NC_CACHE[key]

    in_maps = []
    for c in range(N_CORES):
        sl = slice(c * bc, (c + 1) * bc)
        xd_bf = np.ascontiguousarray(drug[sl].T).astype(ml_dtypes.bfloat16)
        xm_bf = np.ascontiguousarray(micro[sl].T).astype(ml_dtypes.bfloat16)
        mx = np.stack([xd_bf.astype(np.float32).mean(0),
                       xm_bf.astype(np.float32).mean(0)])[None]
        m = {
            "xd": xd_bf, "xm": xm_bf,
            "mx": np.ascontiguousarray(mx).astype(ml_dtypes.bfloat16),
            "a_dm": a_dm, "a_md": a_md, "w1": w1f, "w2": w2f,
        }
        if flags[0]:
            m["c_dm"] = c_dm
        if flags[1]:
            m["c_md"] = c_md
        if flags[2]:
            m["b1"] = b1f
        if flags[3]:
            m["b2"] = b2
        if flags[4]:
            m["g_o"] = g_o
            m["b_o"] = b_o
        in_maps.append(m)

    res = run_bass_kernel_spmd(nc, in_maps, list(range(N_CORES)))
    LAST_RESULTS = res

    out = np.empty((b, D), np.float32)
    for c in range(N_CORES):
        out[c * bc:(c + 1) * bc] = res.results[c]["o"].T
    return out


# revision 58
# speedup vs baseline: 1.0207x; 1.0207x over previous
"""CrossAttentionFusion forward on 8 Trainium2 NeuronCores (pure data parallel).

Math folded on host (seq-len-1 MHA == two chained linears):
  d_att = micro @ A_dm + c_dm,  A_dm = Wv_dm.T @ Wout_dm.T
  m_att = drug  @ A_md + c_md
  u = drug + d_att ; w = micro + m_att
  xu = (u - mu)/sd ; xw likewise        (LN affine folded into W1)
  h1 = gelu([xu, xw] @ W1f + b1f),  W1f = (ffn_w1 * g_cat).T
  h2 = h1 @ W2f + b2,               W2f = ffn_w2.T
  out = ((h2 - mu)/sd) * g_out + b_out

Device layout: feature-major [feat(partition), batch(free)], batch sharded
across 8 cores, NB=256 batch columns per tile (32 tiles/core).

Software pipeline per iteration t (PE program order):
  attn(t) -> out-LN(t-2) -> FFN1(t-1) -> LN-stats(t) -> FFN2(t-1)
so the cross-engine LayerNorm stat chains (PE stats -> DVE smalls ->
GpSimd partition-broadcast -> DVE normalize) always have ~8us of
independent PE work in front of their consumers.  LN mean/rstd are
broadcast across partitions by GpSimd (partition_broadcast), not by PE
matmuls, freeing PE cycles and PSUM banks.
"""

import sys

if "/opt/trn_rl_repo" not in sys.path:
    sys.path.insert(0, "/opt/trn_rl_repo")

from contextlib import ExitStack

import ml_dtypes
import numpy as np

import concourse.bass as bass  # noqa: F401  (registers mybir lowering hooks)
import concourse.tile as tile
from concourse import bacc, mybir
from concourse.bass import ts, broadcast_tensor_aps
from concourse.bass_utils import run_bass_kernel_spmd

F32 = mybir.dt.float32
BF16 = mybir.dt.bfloat16
FP8 = mybir.dt.float8e4
DR = mybir.MatmulPerfMode.DoubleRow
ACT = mybir.ActivationFunctionType

P = 128
D = 384
KD = D // P          # 3
DH = 2 * D           # 768
KH = DH // P         # 6
DF = 4 * D           # 1536
KF = DF // P         # 12
EPS = 1e-5
N_CORES = 8
B_FULL = 65536
BC = B_FULL // N_CORES   # 8192 rows per core
NB = 512                 # batch columns per on-chip tile
NT = BC // NB            # 32 tiles per core

_NC_CACHE = {}
LAST_RESULTS = None      # BassKernelResults of the most recent kernel() call


def _build_nc(bc, flags):
    use_c_dm, use_c_md, use_b1, use_b2, use_affine = flags
    nt = bc // NB
    nc = bacc.Bacc("TRN2", target_bir_lowering=False, debug=False,
                   num_devices=N_CORES)

    xd_d = nc.dram_tensor("xd", [D, bc], BF16, kind="ExternalInput")
    xm_d = nc.dram_tensor("xm", [D, bc], BF16, kind="ExternalInput")
    mx_d = nc.dram_tensor("mx", [1, 2, bc], BF16, kind="ExternalInput")
    a_dm_d = nc.dram_tensor("a_dm", [D, D], BF16, kind="ExternalInput")
    a_md_d = nc.dram_tensor("a_md", [D, D], BF16, kind="ExternalInput")
    w1_d = nc.dram_tensor("w1", [DH, DF], BF16, kind="ExternalInput")
    w2_d = nc.dram_tensor("w2", [DF, D], BF16, kind="ExternalInput")
    c_dm_d = nc.dram_tensor("c_dm", [D], F32, kind="ExternalInput") if use_c_dm else None
    c_md_d = nc.dram_tensor("c_md", [D], F32, kind="ExternalInput") if use_c_md else None
    b1_d = nc.dram_tensor("b1", [DF], F32, kind="ExternalInput") if use_b1 else None
    b2_d = nc.dram_tensor("b2", [D], F32, kind="ExternalInput") if use_b2 else None
    g_o_d = nc.dram_tensor("g_o", [D], F32, kind="ExternalInput") if use_affine else None
    b_o_d = nc.dram_tensor("b_o", [D], F32, kind="ExternalInput") if use_affine else None
    o_d = nc.dram_tensor("o", [D, bc], BF16, kind="ExternalOutput")

    xd_r = xd_d.ap().rearrange("(k p) n -> p k n", p=P)
    xm_r = xm_d.ap().rearrange("(k p) n -> p k n", p=P)
    o_r = o_d.ap().rearrange("(k p) n -> p k n", p=P)

    with tile.TileContext(nc) as tc, ExitStack() as ctx:
        wp = ctx.enter_context(tc.tile_pool(name="wts", bufs=1))
        xp = ctx.enter_context(tc.tile_pool(name="x", bufs=4))
        up = ctx.enter_context(tc.tile_pool(name="u", bufs=2))
        sqp = ctx.enter_context(tc.tile_pool(name="sq", bufs=2))
        xhp = ctx.enter_context(tc.tile_pool(name="xh", bufs=2))
        h1p = ctx.enter_context(tc.tile_pool(name="h1", bufs=2))
        h2p = ctx.enter_context(tc.tile_pool(name="h2", bufs=3))
        op_ = ctx.enter_context(tc.tile_pool(name="o", bufs=2))
        stp = ctx.enter_context(tc.tile_pool(name="st", bufs=2))
        bcp = ctx.enter_context(tc.tile_pool(name="bc", bufs=2))
        # PSUM: 6 single-bank [P,NB] tiles for all matmul outputs
        # (attn 6, ffn1 12, ffn2 3 allocs/iter) + 2 stats banks = 8.
        pp = ctx.enter_context(tc.tile_pool(name="pp", bufs=6, space="PSUM"))
        pst = ctx.enter_context(tc.tile_pool(name="pst", bufs=1, space="PSUM"))

        a_dm_sb = wp.tile([P, KD, D], BF16)
        nc.gpsimd.dma_start(a_dm_sb[:], a_dm_d.ap().rearrange("(k p) m -> p k m", p=P))
        a_md_sb = wp.tile([P, KD, D], BF16)
        nc.gpsimd.dma_start(a_md_sb[:], a_md_d.ap().rearrange("(k p) m -> p k m", p=P))
        # w1/w2 are large (~3.5MB); their DMA emission is deferred into the
        # loop (t==0 tail) so the first tile's mean-broadcast isn't queued
        # behind them on the GpSimd queue.
        w1_sb = wp.tile([P, KH, DF], BF16)
        w2_sb = wp.tile([P, KF, D], BF16)

        def emit_big_weight_dmas():
            nc.gpsimd.dma_start(w1_sb[:], w1_d.ap().rearrange("(k p) m -> p k m", p=P))
            nc.gpsimd.dma_start(w2_sb[:], w2_d.ap().rearrange("(k p) m -> p k m", p=P))

        ones16 = wp.tile([P, 1], BF16)
        nc.vector.memset(ones16[:], 1.0)
        # fp8 DoubleRow "ones" weights for the sum-of-squares stat matmuls:
        # lhsT [P, 2(k-pair), 16(M padded for the 16B step rule)], col 0 = 1.
        ones8 = wp.tile([P, 2, 16], FP8)
        nc.vector.memset(ones8[:], 0.0)
        nc.vector.memset(ones8[:, :, 0:1], 1.0)
        ones8r = wp.tile([P, 1], FP8)
        nc.vector.memset(ones8r[:], 1.0)
        eps_sb = wp.tile([1, 1], F32)
        nc.vector.memset(eps_sb[:], EPS)

        def vec_const(dram, nk, tag):
            t = wp.tile([P, nk], F32, tag=tag)
            nc.gpsimd.dma_start(t[:], dram.ap().rearrange("(k p) -> p k", p=P))
            return t

        c_dm_sb = vec_const(c_dm_d, KD, "c_dm") if use_c_dm else None
        c_md_sb = vec_const(c_md_d, KD, "c_md") if use_c_md else None
        b1_sb = vec_const(b1_d, KF, "b1") if use_b1 else None
        b2_sb = vec_const(b2_d, KD, "b2") if use_b2 else None
        g_o_sb = vec_const(g_o_d, KD, "g_o") if use_affine else None
        b_o_sb = vec_const(b_o_d, KD, "b_o") if use_affine else None

        # Persistent stats PSUM: two [16,NB] slots (2 banks at NB=512; 16
        # partitions because fp8-DoubleRow output spans lhsT.free/2 = 16
        # rows, of which row 0 carries the sum).  Disjoint uses per
        # iteration: {ssq_o} early, {ssq_u, ssq_w} late.
        stat_ps = pst.tile([16, 2, NB], F32)

        def ssq_mms(slot, sq):
            """sum over 3 k-slots of sq (fp8): 1 DoubleRow + 1 regular MM."""
            nc.tensor.matmul(stat_ps[0:16, slot, :], ones8[:],
                             sq[:, 0:2, :], start=True, stop=False,
                             perf_mode=DR, skip_group_check=True)
            nc.tensor.matmul(stat_ps[0:1, slot, :], ones8r[:],
                             sq[:, 2, :], start=False, stop=True,
                             skip_group_check=True)

        def bsub(out_ap, a_ap, b_ap):
            a2, b2_ = broadcast_tensor_aps(a_ap, b_ap)
            o2, _ = broadcast_tensor_aps(out_ap, b_ap)
            nc.vector.tensor_sub(o2, a2, b2_)

        def bmul(out_ap, a_ap, b_ap):
            a2, b2_ = broadcast_tensor_aps(a_ap, b_ap)
            o2, _ = broadcast_tensor_aps(out_ap, b_ap)
            nc.vector.tensor_mul(o2, a2, b2_)

        # LN structure: attention matrices are row-mean-centered on the host
        # (A' = A - rowmean(A)), so mean(u) == colmean(xd) exactly; the means
        # themselves are precomputed on the host and DMA'd in.  Residuals are
        # centered BEFORE the attention output lands, the Squares of the
        # centered sums give the variance directly, and the only
        # latency-critical chain is
        #   ssq-matmuls -> 1/D scale -> Sqrt -> recip -> cast -> inv bcast
        #   -> 2 muls -> xh
        # ONE Sqrt instruction per iteration covers slots {0,1}=u/w(t) and
        # {2}=out-LN(t-2) (single Gelu<->Sqrt table round-trip).
        def rsqrt_chain(var3, lo, hi):
            sd = stp.tile([1, 3, NB], F32, tag="sd")
            nc.scalar.activation(sd[:, lo:hi, :], var3[:, lo:hi, :], ACT.Sqrt,
                                 bias=eps_sb[:])
            inv = stp.tile([1, 3, NB], F32, tag="iv")
            nc.vector.reciprocal_approx_fast(inv[:, lo:hi, :], sd[:, lo:hi, :])
            inv16 = stp.tile([1, 3, NB], BF16, tag="iv16")
            nc.vector.tensor_copy(inv16[:, lo:hi, :], inv[:, lo:hi, :])
            bc_uw = bc_o = None
            if lo == 0:
                bc_uw = bcp.tile([P, 2, NB], BF16, tag="ivbuw")
                nc.gpsimd.partition_broadcast(bc_uw[:], inv16[:, 0:2, :])
            if hi == 3:
                bc_o = bcp.tile([P, 1, NB], BF16, tag="ivbo")
                nc.gpsimd.partition_broadcast(bc_o[:], inv16[:, 2:3, :])
            return bc_uw, bc_o

        state = {}

        def emit_means(t, sl):
            xd = xp.tile([P, KD, NB], BF16, tag="xd")
            nc.sync.dma_start(xd[:], xd_r[:, :, sl])
            xm = xp.tile([P, KD, NB], BF16, tag="xm")
            nc.sync.dma_start(xm[:], xm_r[:, :, sl])
            mu_uw = stp.tile([1, 2, NB], BF16, tag="mu_uw")
            nc.sync.dma_start(mu_uw[:], mx_d.ap()[:, :, sl])
            mu_bc = bcp.tile([P, 2, NB], BF16, tag="mubuw")
            nc.gpsimd.partition_broadcast(mu_bc[:], mu_uw[:])
            # centered residuals, ready before the attention output lands
            xd_c = up.tile([P, KD, NB], BF16, tag="xdc")
            bsub(xd_c[:], xd[:], mu_bc[:, 0:1, :])
            xm_c = up.tile([P, KD, NB], BF16, tag="xmc")
            bsub(xm_c[:], xm[:], mu_bc[:, 1:2, :])
            state[t] = {"xd": xd, "xm": xm, "xd_c": xd_c, "xm_c": xm_c}

        def emit_attn(t):
            st_ = state[t]
            xd, xm = st_["xd"], st_["xm"]

            def attn(a_sb, rhs, res_c, c_sb, tag):
                v = up.tile([P, KD, NB], BF16, tag=tag)
                for m in range(KD):
                    ps = pp.tile([P, NB], F32, tag="pp")
                    for k in range(KD):
                        nc.tensor.matmul(ps[:],
                                         a_sb[:, k, ts(m, P)],
                                         rhs[:, k, :],
                                         start=(k == 0), stop=(k == KD - 1))
                    nc.vector.tensor_add(v[:, m, :], ps[:], res_c[:, m, :])
                    if c_sb is not None:
                        nc.vector.tensor_scalar_add(v[:, m, :], v[:, m, :],
                                                    c_sb[:, m:m + 1])
                sq = sqp.tile([P, KD, NB], FP8, tag="sq" + tag)
                nc.scalar.activation(sq[:], v[:], ACT.Square)
                return v, sq

            u, sq_u = attn(a_dm_sb, xm, st_["xd_c"], c_dm_sb, "u")
            w, sq_w = attn(a_md_sb, xd, st_["xm_c"], c_md_sb, "w")
            st_.update(u=u, w=w, sq_u=sq_u, sq_w=sq_w)

        def emit_uw_ssq(t, var3):
            st_ = state[t]
            ssq_mms(0, st_["sq_u"])
            ssq_mms(1, st_["sq_w"])
            nc.vector.tensor_scalar_mul(var3[:, 0:2, :], stat_ps[0:1, 0:2, :],
                                        float(1.0 / D))

        def emit_norm_uw(t, inv_bc):
            st_ = state[t]
            xh = xhp.tile([P, KH, NB], BF16, tag="xh")
            bmul(xh[:, 0:KD, :], st_["u"][:], inv_bc[:, 0:1, :])
            bmul(xh[:, KD:KH, :], st_["w"][:], inv_bc[:, 1:2, :])
            st_["xh"] = xh

        def emit_ffn1(t):
            st_ = state[t]
            xh = st_["xh"]
            h1 = h1p.tile([P, KF, NB], BF16, tag="h1")
            for m in range(KF):
                ps = pp.tile([P, NB], F32, tag="pp")
                for k in range(KH):
                    nc.tensor.matmul(ps[:],
                                     w1_sb[:, k, ts(m, P)],
                                     xh[:, k, :],
                                     start=(k == 0), stop=(k == KH - 1))
                if use_b1:
                    nc.scalar.activation(h1[:, m, :], ps[:],
                                         ACT.Gelu, bias=b1_sb[:, m:m + 1])
                else:
                    nc.scalar.activation(h1[:, m, :], ps[:], ACT.Gelu)
            st_["h1"] = h1

        def emit_ffn2(t):
            st_ = state[t]
            h1 = st_["h1"]
            h2 = h2p.tile([P, KD, NB], BF16, tag="h2")
            for m in range(KD):
                ps = pp.tile([P, NB], F32, tag="pp")
                for k in range(KF):
                    nc.tensor.matmul(ps[:], w2_sb[:, k, ts(m, P)],
                                     h1[:, k, :],
                                     start=(k == 0), stop=(k == KF - 1))
                if use_b2:
                    nc.scalar.activation(h2[:, m, :], ps[:],
                                         ACT.Identity, bias=b2_sb[:, m:m + 1])
                else:
                    nc.vector.tensor_copy(h2[:, m, :], ps[:])
            st_["h2"] = h2

        def emit_outln_stats(t, var3):
            # W2 is row-mean-centered on the host, so mean(h2) == 0 exactly:
            # no mean matmuls / broadcast / subtraction needed for the out-LN.
            st_ = state[t]
            h2 = st_["h2"]
            sq_o = sqp.tile([P, KD, NB], FP8, tag="sqo")
            nc.scalar.activation(sq_o[:], h2[:], ACT.Square)
            ssq_mms(1, sq_o)
            nc.vector.tensor_scalar_mul(var3[:, 2, :], stat_ps[0:1, 1, :],
                                        float(1.0 / D))

        def emit_outln_norm_store(t, inv_bo, sl):
            st_ = state.pop(t)
            h2 = st_["h2"]
            o = op_.tile([P, KD, NB], BF16, tag="o")
            bmul(o[:], h2[:], inv_bo[:])
            if use_affine:
                for k in range(KD):
                    nc.vector.tensor_scalar(o[:, k, :], o[:, k, :],
                                            g_o_sb[:, k:k + 1],
                                            b_o_sb[:, k:k + 1],
                                            mybir.AluOpType.mult,
                                            mybir.AluOpType.add)
            nc.sync.dma_start(o_r[:, :, sl], o[:])

        for t in range(nt + 2):
            sl = slice(t * NB, (t + 1) * NB)
            lo = 0 if t < nt else 2
            hi = 3 if t >= 2 else 2
            var3 = stp.tile([1, 3, NB], F32, tag="var3")
            if t < nt:
                emit_means(t, sl)
                emit_attn(t)
            if t == 0:
                emit_big_weight_dmas()
            if t >= 2:
                emit_outln_stats(t - 2, var3)
            if 1 <= t <= nt:
                emit_ffn1(t - 1)
            if t < nt:
                emit_uw_ssq(t, var3)
            bc_uw, bc_o = rsqrt_chain(var3, lo, hi)
            if t < nt:
                emit_norm_uw(t, bc_uw)
            if t >= 2:
                osl = slice((t - 2) * NB, (t - 1) * NB)
                emit_outln_norm_store(t - 2, bc_o, osl)
            if 1 <= t <= nt:
                emit_ffn2(t - 1)

    nc.compile()
    return nc


def kernel(**inputs) -> np.ndarray:
    global LAST_RESULTS
    f = lambda k: np.asarray(inputs[k], np.float32)

    drug = f("drug_emb")
    micro = f("micro_emb")
    b = drug.shape[0]
    bc = b // N_CORES
    assert b % (N_CORES * NB) == 0

    # ---- host-side weight folding ----
    wv_dm, bv_dm = f("dm_in_w")[2 * D:], f("dm_in_b")[2 * D:]
    wv_md, bv_md = f("md_in_w")[2 * D:], f("md_in_b")[2 * D:]
    # Row-mean-center the attention matrices (and mean-center the bias
    # vectors) so that on-device mean(u) == colmean(xd): the LN mean then
    # only depends on the raw input, not on the attention output.
    a_dm = wv_dm.T @ f("dm_out_w").T
    a_dm = np.ascontiguousarray(a_dm - a_dm.mean(1, keepdims=True)).astype(ml_dtypes.bfloat16)
    c_dm = bv_dm @ f("dm_out_w").T + f("dm_out_b")
    c_dm = c_dm - c_dm.mean()
    a_md = wv_md.T @ f("md_out_w").T
    a_md = np.ascontiguousarray(a_md - a_md.mean(1, keepdims=True)).astype(ml_dtypes.bfloat16)
    c_md = bv_md @ f("md_out_w").T + f("md_out_b")
    c_md = c_md - c_md.mean()
    g_cat = np.concatenate([f("norm_d_g"), f("norm_m_g")])
    b_cat = np.concatenate([f("norm_d_b"), f("norm_m_b")])
    w1f = np.ascontiguousarray((f("ffn_w1") * g_cat[None, :]).T).astype(ml_dtypes.bfloat16)
    b1f = f("ffn_b1") + b_cat @ f("ffn_w1").T
    # Row-mean-center W2 (and mean-center b2) so mean(h2) == 0 exactly and
    # the out-LN needs no mean computation on device.
    w2f = f("ffn_w2").T
    w2f = np.ascontiguousarray(w2f - w2f.mean(1, keepdims=True)).astype(ml_dtypes.bfloat16)
    b2 = f("ffn_b2")
    b2 = b2 - b2.mean()
    g_o, b_o = f("norm_out_g"), f("norm_out_b")

    flags = (bool(np.any(c_dm)), bool(np.any(c_md)), bool(np.any(b1f)),
             bool(np.any(b2)), bool(np.any(g_o != 1.0) or np.any(b_o)))

    key = (bc, NB, flags)
    if key not in _NC_CACHE:
        _NC_CACHE[key] = _build_nc(bc, flags)
    nc = _NC_CACHE[key]

    in_maps = []
    for c in range(N_CORES):
        sl = slice(c * bc, (c + 1) * bc)
        xd_bf = np.ascontiguousarray(drug[sl].T).astype(ml_dtypes.bfloat16)
        xm_bf = np.ascontiguousarray(micro[sl].T).astype(ml_dtypes.bfloat16)
        mx = np.stack([xd_bf.astype(np.float32).mean(0),
                       xm_bf.astype(np.float32).mean(0)])[None]
        m = {
            "xd": xd_bf, "xm": xm_bf,
            "mx": np.ascontiguousarray(mx).astype(ml_dtypes.bfloat16),
            "a_dm": a_dm, "a_md": a_md, "w1": w1f, "w2": w2f,
        }
        if flags[0]:
            m["c_dm"] = c_dm
        if flags[1]:
            m["c_md"] = c_md
        if flags[2]:
            m["b1"] = b1f
        if flags[3]:
            m["b2"] = b2
        if flags[4]:
            m["g_o"] = g_o
            m["b_o"] = b_o
        in_maps.append(m)

    res = run_bass_kernel_spmd(nc, in_maps, list(range(N_CORES)))
    LAST_RESULTS = res

    out = np.empty((b, D), np.float32)
    for c in range(N_CORES):
        out[c * bc:(c + 1) * bc] = res.results[c]["o"].T
    return out
